# revision 1
# baseline (speedup 1.0000x reference)
"""MoE layer (dense all-expert routing) Trainium2 Bass kernel.

Problem: x[4,2048,1024] f32, gate_w[1024,8], gate_b[8], expert_w[8,1024,1024].
  gate = softmax(x @ gate_w + gate_b)                  # [B,S,E]
  out  = einsum('bse,bseo->bso', gate, einsum('bsi,eio->bseo', x, expert_w))

Sharding: data-parallel over tokens. 8192 tokens split into 8 shards of 1024;
each core computes its shard against all 8 experts (weights replicated).
No collectives; host concatenates shard outputs.

Per-core kernel:
  - all matmuls bf16 with f32 PSUM accumulation (rel err ~3e-3)
  - gate logits on PE in [token, expert] orientation; gate_b added via a
    K=1 ones-matmul into the same PSUM accumulation group; softmax is then
    all free-dim ops (exp w/ accum_out, reciprocal, scale)
  - gate uses its own 1-slot PSUM tag; main loop uses 7 slots so the first
    expert's matmuls start as soon as x/W tiles land (no gate dependency)
  - main loop: per (n-half, expert) one 1MiB weight DMA, 64 matmuls,
    then per m one fused DVE op: acc = (psum * g[:,e]) + acc
  - input DMAs split across both HWDGE rings (sync + scalar) for latency
"""

import numpy as np
import ml_dtypes
from contextlib import ExitStack

import concourse.bacc as bacc
import concourse.bass as bass
import concourse.mybir as mybir
import concourse.tile as tile

BF16 = mybir.dt.bfloat16
F32 = mybir.dt.float32

P = 128  # partitions


def build_moe_nc(T=1024, D=1024, O=1024, E=8, NO=512, w_bufs=2, acc_bufs=16):
    """Build the per-core Bass program.

    T: tokens per core, D: d_in, O: d_out, E: experts, NO: d_out tile (<=512).
    """
    KT = D // P   # k tiles (contraction)
    MT = T // P   # token tiles
    NT = O // NO  # d_out tiles

    nc = bacc.Bacc("TRN2", target_bir_lowering=False, debug=False)
    xT_d = nc.dram_tensor("xT", [D, T], BF16, kind="ExternalInput")
    w_d = nc.dram_tensor("w", [E, D, O], BF16, kind="ExternalInput")
    # gwt[p, k*E+e] = gate_w[k*128+p, e]  (host pre-tiled, contiguous DMA)
    gwt_d = nc.dram_tensor("gwt", [P, KT * E], BF16, kind="ExternalInput")
    gb_d = nc.dram_tensor("gb", [1, E], BF16, kind="ExternalInput")
    out_d = nc.dram_tensor("out", [T, O], F32, kind="ExternalOutput")

    with tile.TileContext(nc) as tc:
        with ExitStack() as ctx:
            singles = ctx.enter_context(tc.tile_pool(name="singles", bufs=1))
            wpool = ctx.enter_context(tc.tile_pool(name="w", bufs=w_bufs))
            accp = ctx.enter_context(tc.tile_pool(name="acc", bufs=acc_bufs))
            gpool = ctx.enter_context(tc.tile_pool(name="gate", bufs=1))
            ps = ctx.enter_context(tc.tile_pool(name="ps", bufs=7, space="PSUM"))

            # ---- resident loads -------------------------------------------
            # The head is DMA-latency-bound: the PE needs (xT[k], W[e0,k])
            # PAIRS in k order. Interleave them strictly, alternating the
            # two HWDGE rings so trigger serialization doesn't bind. Loads
            # for k and the first expert's weight chunk are issued together
            # below (w0 chunks into wt0 allocated here).
            ones_t = singles.tile([1, P], BF16, tag="ones")
            nc.vector.memset(ones_t, 1.0)

            # HAM warm-up: dummy matmuls on a memset tile keep the PE busy
            # while the first input DMAs are in flight, so the clock gate is
            # at 8/8 when real work starts. Results are never read.
            warm = singles.tile([P, NO], BF16, tag="warm")
            nc.vector.memset(warm, 0.0)
            psw = ps.tile([P, NO], F32, tag="psg", bufs=1, name="psw")
            for j in range(12):
                nc.tensor.matmul(
                    psw, lhsT=warm[:, 0:P], rhs=warm,
                    start=(j == 0), stop=(j == 11),
                )

            # Head loads: (xT, W[e0]) k-chunks, single-k first (so the PE
            # can start at the earliest possible moment), k-pairs after,
            # alternating rings so the k-ordered consumption is fed in
            # arrival order. Per-ring FIFO = natural priority; steady-state
            # weights go via SWDGE (gpsimd) below, so the HWDGE rings never
            # convoy on sem-lane reuse.
            # (xT[k], W0[k]) chunks in k order: single-k first (earliest PE
            # start), k-pairs after, alternating the two HWDGE rings; the
            # per-ring FIFO gives natural priority
            wt0 = wpool.tile([P, KT, NO], BF16, tag="w", name="wt0")
            w0_src = w_d[0, :, 0:NO].rearrange("(k p) o -> p k o", p=P)
            chunks = [(0, 1, nc.sync), (1, 1, nc.scalar)]
            k = 2
            while k < KT:
                nk = min(2, KT - k)
                eng = nc.sync if len(chunks) % 2 == 0 else nc.scalar
                chunks.append((k, nk, eng))
                k += nk
            xparts = {}
            for (kc, nk, eng) in chunks:
                t = singles.tile(
                    [P, nk, T], BF16, tag=f"xT{kc}", name=f"xc{kc}"
                )
                eng.dma_start(
                    out=t,
                    in_=xT_d[kc * P:(kc + nk) * P, :].rearrange(
                        "(k p) t -> p k t", p=P
                    ),
                )
                eng.dma_start(
                    out=wt0[:, kc:kc + nk, :], in_=w0_src[:, kc:kc + nk, :]
                )
                for i in range(nk):
                    xparts[kc + i] = (t, i)

            def xT(k):
                t, i = xparts[k]
                return t[:, i, :]

            # gate consts late on the scalar ring — only needed once every
            # xT chunk has landed anyway
            gb_sb = singles.tile([1, E], BF16, tag="gb")
            nc.scalar.dma_start(out=gb_sb, in_=gb_d[:, :])
            gw_t = singles.tile([P, KT, E], BF16, tag="gw")
            nc.scalar.dma_start(
                out=gw_t, in_=gwt_d[:, :].rearrange("p (k e) -> p k e", e=E)
            )

            # ---- gate: own 1-slot PSUM tag, serialized m-groups -----------
            # Emitted before the main loop; its matmuls depend on all xT
            # chunks so they fill PE gaps while the main stream runs.
            g_sb = [None] * MT

            def emit_gate():
                for m in range(MT):
                    psg = ps.tile([P, E], F32, tag="psg", bufs=1, name=f"psg{m}")
                    nc.tensor.matmul(
                        psg, lhsT=ones_t, rhs=gb_sb, start=True, stop=False
                    )
                    for k in range(KT):
                        nc.tensor.matmul(
                            psg,
                            lhsT=xT(k)[:, m * P:(m + 1) * P],
                            rhs=gw_t[:, k, :],
                            start=False,
                            stop=(k == KT - 1),
                        )
                    p_t = gpool.tile([P, E], F32, tag=f"p{m}", name=f"p{m}")
                    s_t = gpool.tile([P, 1], F32, tag=f"s{m}", name=f"s{m}")
                    # exp(logits); |logits| <~ 3 so no max-subtraction needed
                    nc.scalar.activation(
                        p_t, psg, mybir.ActivationFunctionType.Exp,
                        accum_out=s_t,
                    )
                    rs_t = gpool.tile([P, 1], F32, tag=f"rs{m}", name=f"rs{m}")
                    nc.vector.reciprocal(rs_t, s_t)
                    g_t = gpool.tile([P, E], F32, tag=f"g{m}", name=f"g{m}")
                    nc.vector.tensor_scalar_mul(g_t, p_t, rs_t)
                    g_sb[m] = g_t

            emit_gate()

            # ---- main: all-expert GEMM + fused gate combine ---------------
            for n in range(NT):
                acc = [None] * MT
                for e in range(E):
                    # one 1MiB DMA per (n, e): all k-tiles of this d_out
                    # slice. (n==0, e==0) was loaded k-granular at the head.
                    if n == 0 and e == 0:
                        wt = wt0
                    else:
                        # e1 rides the sync HWDGE ring behind the head
                        # chunks (ring FIFO = natural priority); the rest go
                        # SWDGE so the HWDGE rings never convoy on sem lanes
                        wt = wpool.tile([P, KT, NO], BF16, tag="w")
                        eng = nc.sync if (n == 0 and e == 1) else nc.gpsimd
                        eng.dma_start(
                            out=wt,
                            in_=w_d[e, :, n * NO:(n + 1) * NO].rearrange(
                                "(k p) o -> p k o", p=P
                            ),
                        )
                    # Expert 0 (head, DMA-paced): k-outer so the PE can
                    # consume each arriving k chunk across all m groups.
                    # Experts 1+: m-outer — each PSUM group is 8 consecutive
                    # matmuls, slots cycle fast, and the per-m combine +
                    # output DMA spread across the stream.
                    if n == 0 and e == 0:
                        psy_l = [None] * MT
                        for k in range(KT):
                            for m in range(MT - 1):
                                if k == 0:
                                    psy_l[m] = ps.tile(
                                        [P, NO], F32, tag="ps", name=f"psk{m}"
                                    )
                                nc.tensor.matmul(
                                    psy_l[m],
                                    lhsT=xT(k)[:, m * P:(m + 1) * P],
                                    rhs=wt[:, k, :],
                                    start=(k == 0),
                                    stop=(k == KT - 1),
                                )
                        psy_l[MT - 1] = ps.tile(
                            [P, NO], F32, tag="ps", name="psk_last"
                        )
                        for k in range(KT):
                            nc.tensor.matmul(
                                psy_l[MT - 1],
                                lhsT=xT(k)[:, (MT - 1) * P:MT * P],
                                rhs=wt[:, k, :],
                                start=(k == 0),
                                stop=(k == KT - 1),
                            )
                        for m in range(MT):
                            acc[m] = accp.tile(
                                [P, NO], F32, tag="acc", name=f"acc{m}"
                            )
                            nc.vector.tensor_copy(acc[m], psy_l[m])
                            nc.vector.tensor_scalar_mul(
                                acc[m], acc[m], g_sb[m][:, 0:1]
                            )
                        continue
                    for m in range(MT):
                        if n == NT - 1 and e == E - 1 and m == MT - 1:
                            # very last tile: two column-half PSUM groups so
                            # the first half's combine + output DMA overlap
                            # the second half's matmuls (shorter tail)
                            NH = NO // 2
                            for h in range(2):
                                psy_h = ps.tile(
                                    [P, NH], F32, tag="ps", name=f"psyh{h}"
                                )
                                for k in range(KT):
                                    nc.tensor.matmul(
                                        psy_h,
                                        lhsT=xT(k)[:, m * P:(m + 1) * P],
                                        rhs=wt[:, k, h * NH:(h + 1) * NH],
                                        start=(k == 0),
                                        stop=(k == KT - 1),
                                    )
                                nc.vector.scalar_tensor_tensor(
                                    out=acc[m][:, h * NH:(h + 1) * NH],
                                    in0=psy_h,
                                    scalar=g_sb[m][:, e:e + 1],
                                    in1=acc[m][:, h * NH:(h + 1) * NH],
                                    op0=mybir.AluOpType.mult,
                                    op1=mybir.AluOpType.add,
                                )
                                eng = nc.scalar if h == 0 else nc.sync
                                eng.dma_start(
                                    out=out_d[
                                        m * P:(m + 1) * P,
                                        n * NO + h * NH:n * NO + (h + 1) * NH,
                                    ],
                                    in_=acc[m][:, h * NH:(h + 1) * NH],
                                )
                            continue
                        psy = ps.tile([P, NO], F32, tag="ps", name=f"psy{m}")
                        for k in range(KT):
                            nc.tensor.matmul(
                                psy,
                                lhsT=xT(k)[:, m * P:(m + 1) * P],
                                rhs=wt[:, k, :],
                                start=(k == 0),
                                stop=(k == KT - 1),
                            )
                        if e == 0:
                            # init acc with an unscaled copy (no gate dep —
                            # frees the PSUM slot even if the gate is still
                            # running), then fold g0 in as a separate op
                            acc[m] = accp.tile(
                                [P, NO], F32, tag="acc", name=f"acc{m}"
                            )
                            nc.vector.tensor_copy(acc[m], psy)
                            nc.vector.tensor_scalar_mul(
                                acc[m], acc[m], g_sb[m][:, 0:1]
                            )
                        else:
                            nc.vector.scalar_tensor_tensor(
                                out=acc[m],
                                in0=psy,
                                scalar=g_sb[m][:, e:e + 1],
                                in1=acc[m],
                                op0=mybir.AluOpType.mult,
                                op1=mybir.AluOpType.add,
                            )
                        if e == E - 1:
                            eng = nc.sync if m % 2 == 0 else nc.scalar
                            eng.dma_start(
                                out=out_d[
                                    m * P:(m + 1) * P, n * NO:(n + 1) * NO
                                ],
                                in_=acc[m],
                            )
    nc.compile()
    return nc


# ---------------------------------------------------------------------------
# Host wrapper: full inputs -> shard -> run SPMD on 8 cores -> gather
# ---------------------------------------------------------------------------

N_CORES = 8
_B, _S, _DIN, _DOUT, _E = 4, 2048, 1024, 1024, 8


def _host_gwt(gate_w):
    """[D, E] -> [128, KT*E] with gwt[p, k*E+e] = gate_w[k*128+p, e]."""
    D, E = gate_w.shape
    kt = D // P
    return np.ascontiguousarray(
        gate_w.reshape(kt, P, E).transpose(1, 0, 2).reshape(P, kt * E)
    )


LAST_RESULTS = None  # BassKernelResults of the most recent run (for profiling)


def kernel(x, gate_w, gate_b, expert_w, _trace=False):
    global LAST_RESULTS
    from concourse.bass_utils import run_bass_kernel_spmd

    x = np.asarray(x)
    tokens = x.reshape(-1, _DIN)  # [8192, 1024]
    n_tok = tokens.shape[0]
    tpc = n_tok // N_CORES  # tokens per core

    w_bf = np.asarray(expert_w, dtype=ml_dtypes.bfloat16)
    gwt_bf = _host_gwt(np.asarray(gate_w)).astype(ml_dtypes.bfloat16)
    gb_bf = np.asarray(gate_b, dtype=np.float32).reshape(1, _E).astype(
        ml_dtypes.bfloat16
    )

    in_maps = []
    for c in range(N_CORES):
        shard = tokens[c * tpc:(c + 1) * tpc]  # [1024, 1024]
        xT = np.ascontiguousarray(shard.T).astype(ml_dtypes.bfloat16)
        in_maps.append({"xT": xT, "w": w_bf, "gwt": gwt_bf, "gb": gb_bf})

    nc = build_moe_nc(T=tpc, D=_DIN, O=_DOUT, E=_E)
    res = run_bass_kernel_spmd(nc, in_maps, list(range(N_CORES)), trace=_trace)
    LAST_RESULTS = res
    outs = [res.results[c]["out"] for c in range(N_CORES)]
    full = np.concatenate(outs, axis=0).astype(np.float32)
    return full.reshape(_B, _S, _DOUT)

